# revision 10
# baseline (speedup 1.0000x reference)
"""Trainium2 Bass kernel for the 3-layer GAT (nn_GAT_56118042689980).

Strategy: destination-node sharding across 8 cores.
  - Host (numpy, data layout only): build W_ext = [W | W@att_src | W@att_dst],
    sort edges by dst, partition by dst range, group into 128-dst blocks x
    128-edge tiles, pad uniformly.
  - Per layer on chip: node phase (sliced matmul x@W_ext -> AllGather full
    table), edge phase (indirect-DMA row gather by src, one-hot sel matmuls
    accumulating weighted sums + softmax denominators in PSUM, selT matmuls
    broadcasting per-dst values back to edges).
"""

import os
import sys
from dataclasses import dataclass

import numpy as np

# ---------------------------------------------------------------- constants

I_ITEMS, U_USERS, F_FETS = 40000, 30000, 30000
N_NODES = 100000
D_EMB = 64
E_EDGES = 1200000
NEG_SLOPE = 0.2
NCORES = 8

SLICE = 12544            # nodes per core; NPAD = 8*12544 = 100352 >= 100000
NPAD = SLICE * NCORES
BLOCKS = 98              # dst blocks of 128 per core (98*128 = 12544)
MCAP = 15                # edge tiles (of 128) per block, uniform padding
TTOT = BLOCKS * MCAP     # edge tiles per core
EPAD = TTOT * 128        # edge slots per core
PAD_NODE = NPAD - 1      # gather index for pad edge slots (a zero row)
PAD_DSTREL = 999.0       # sentinel; is_equal never matches -> zero sel column

# per layer: (F_in, F_out, heads)
LAYERS = [(64, 128, 2), (128, 128, 2), (128, 64, 1)]
XCOLS_MAX = 132          # F_out + 2*H max


def _xcols(fo, h):
    return fo + 2 * h


# ---------------------------------------------------------------- host prep

def build_w_ext(W, att_src, att_dst):
    """W_ext = [W | W @ att_src per head | W @ att_dst per head]  (f32)."""
    fin, fohh = W.shape
    h, c = att_src.shape
    fo = h * c
    assert fohh == fo
    out = np.zeros((fin, fo + 2 * h), np.float32)
    out[:, :fo] = W
    for hh in range(h):
        out[:, fo + hh] = W[:, hh * c:(hh + 1) * c] @ att_src[hh]
        out[:, fo + h + hh] = W[:, hh * c:(hh + 1) * c] @ att_dst[hh]
    return out


def host_prep_edges(edge_index, n_nodes=N_NODES, slice_=SLICE, blocks=BLOCKS,
                    mcap=MCAP, ncores=NCORES):
    """Sort edges (incl. self loops) by dst, partition by dst range, lay out
    into per-core [128, TTOT] slot grids.

    Returns per-core dicts with gsrc (int32 gather idx), dstrel (f32),
    orig (int64 original edge id, -1 for pads)."""
    npad = slice_ * ncores
    ttot = blocks * mcap
    pad_node = npad - 1
    src = np.concatenate([edge_index[0], np.arange(n_nodes, dtype=np.int32)])
    dst = np.concatenate([edge_index[1], np.arange(n_nodes, dtype=np.int32)])
    order = np.argsort(dst, kind="stable")
    src_s = src[order].astype(np.int64)
    dst_s = dst[order].astype(np.int64)

    cores = []
    bounds = np.searchsorted(dst_s, np.arange(ncores + 1) * slice_)
    for k in range(ncores):
        a, b = bounds[k], bounds[k + 1]
        dl = dst_s[a:b] - k * slice_
        blk = dl >> 7
        rel = dl & 127
        # rank of each edge within its block (edges are dst-sorted => block-grouped)
        blk_start = np.searchsorted(blk, np.arange(blocks))
        r = np.arange(b - a) - blk_start[blk]
        assert r.max(initial=0) < mcap * 128, (
            f"core {k}: block overflow {r.max()} >= {mcap * 128}")
        t = r >> 7
        p = r & 127
        g = blk * mcap + t
        gsrc = np.full((128, ttot), pad_node, np.int32)
        dstrel = np.full((128, ttot), PAD_DSTREL, np.float32)
        orig = np.full((128, ttot), -1, np.int64)
        gsrc[p, g] = src_s[a:b]
        dstrel[p, g] = rel
        orig[p, g] = order[a:b]
        cores.append(dict(gsrc=gsrc, dstrel=dstrel, orig=orig))
    return cores


def host_prep_all(inputs, cfg=None, n_real=N_NODES):
    """All host-side arrays: per-core input maps + postprocess info."""
    cfg = cfg or Cfg()
    x0 = np.concatenate(
        [inputs["emb_item"], inputs["emb_user"], inputs["emb_fet"]], axis=0)
    d_emb = x0.shape[1]
    x0p = np.zeros((cfg.npad, d_emb), np.float32)
    x0p[:n_real] = x0

    wexts = [
        build_w_ext(inputs["W1"], inputs["att_src1"], inputs["att_dst1"]),
        build_w_ext(inputs["W2"], inputs["att_src2"], inputs["att_dst2"]),
        build_w_ext(inputs["W3"], inputs["att_src3"], inputs["att_dst3"]),
    ]
    biases = [np.asarray(inputs["b1"], np.float32).reshape(1, -1),
              np.asarray(inputs["b2"], np.float32).reshape(1, -1),
              np.asarray(inputs["b3"], np.float32).reshape(1, -1)]

    edge_cores = host_prep_edges(inputs["edge_index"], n_nodes=n_real,
                                 slice_=cfg.slice_, blocks=cfg.blocks,
                                 mcap=cfg.mcap, ncores=cfg.ncores)

    in_maps = []
    for k in range(cfg.ncores):
        sl = x0p[k * cfg.slice_:(k + 1) * cfg.slice_]
        m = {
            "x0T": np.ascontiguousarray(sl.T),        # [64, SLICE]
            "x0s": np.ascontiguousarray(sl),          # [SLICE, 64]
            "gsrc": edge_cores[k]["gsrc"],            # [128, TTOT] int32
            "dstrel": edge_cores[k]["dstrel"],        # [128, TTOT] f32
            "Wext1": wexts[0], "Wext2": wexts[1], "Wext3": wexts[2],
            "bias1": biases[0], "bias2": biases[1], "bias3": biases[2],
        }
        in_maps.append(m)
    origs = [edge_cores[k]["orig"] for k in range(cfg.ncores)]
    return in_maps, origs


def postprocess(results, origs, cfg=None, n_real=N_NODES, e_real=E_EDGES,
                splits=(I_ITEMS, U_USERS)):
    """Assemble full outputs from per-core results."""
    cfg = cfg or Cfg()
    final = np.concatenate([r["final_out"] for r in results], axis=0)[:n_real]
    ep = e_real + n_real
    alphas = []
    for li, (_, _, h) in enumerate(cfg.layers):
        full = np.zeros((ep, h), np.float32)
        for k in range(cfg.ncores):
            v = results[k][f"alpha{li + 1}"].reshape(128, cfg.ttot, h)
            o = origs[k]
            msk = o >= 0
            full[o[msk]] = v[msk]
        alphas.append(full)
    a, b = splits
    return (final[:a], final[a:a + b],
            final[a + b:], final, alphas[0], alphas[1], alphas[2])


# ---------------------------------------------------------------- bass build

@dataclass
class Cfg:
    slice_: int = SLICE
    blocks: int = BLOCKS
    mcap: int = MCAP
    ncores: int = NCORES
    layers: tuple = ((64, 128, 2), (128, 128, 2), (128, 64, 1))

    @property
    def npad(self):
        return self.slice_ * self.ncores

    @property
    def ttot(self):
        return self.blocks * self.mcap

    @property
    def xcols_max(self):
        return max(_xcols(fo, h) for _, fo, h in self.layers)


def build_nc(cfg: Cfg = Cfg(), debug=False, dump=False):
    import concourse.bass as bass
    import concourse.mybir as mybir
    import concourse.tile as tile
    from concourse import bacc
    from concourse.masks import make_identity

    f32 = mybir.dt.float32
    i32 = mybir.dt.int32
    AT = mybir.ActivationFunctionType
    OP = mybir.AluOpType

    S, B, M, T = cfg.slice_, cfg.blocks, cfg.mcap, cfg.ttot
    NP = cfg.npad
    XM = cfg.xcols_max
    fin1 = cfg.layers[0][0]

    nc = bacc.Bacc("TRN2", target_bir_lowering=False, debug=debug,
                   enable_asserts=False, num_devices=cfg.ncores)

    # ---- I/O
    x0T = nc.dram_tensor("x0T", [fin1, S], f32, kind="ExternalInput")
    x0s = nc.dram_tensor("x0s", [S, fin1], f32, kind="ExternalInput")
    gsrc = nc.dram_tensor("gsrc", [128, T], i32, kind="ExternalInput")
    dstrel = nc.dram_tensor("dstrel", [128, T], f32, kind="ExternalInput")
    wext_d, bias_d = [], []
    for li, (fi, fo, h) in enumerate(cfg.layers):
        wext_d.append(nc.dram_tensor(f"Wext{li + 1}", [fi, _xcols(fo, h)], f32,
                                     kind="ExternalInput"))
        bias_d.append(nc.dram_tensor(f"bias{li + 1}", [1, fo], f32,
                                     kind="ExternalInput"))
    final_out = nc.dram_tensor("final_out", [S, 64], f32, kind="ExternalOutput")
    if dump:
        dmp = {
            "d_iota": nc.dram_tensor("d_iota", [128, 128], f32, kind="ExternalOutput"),
            "d_xsl": nc.dram_tensor("d_xsl", [S, _xcols(cfg.layers[0][1], cfg.layers[0][2])], f32, kind="ExternalOutput"),
            "d_xfull": nc.dram_tensor("d_xfull", [NP, _xcols(cfg.layers[0][1], cfg.layers[0][2])], f32, kind="ExternalOutput"),
            "d_gbuf": nc.dram_tensor("d_gbuf", [128, cfg.mcap * _xcols(cfg.layers[0][1], cfg.layers[0][2])], f32, kind="ExternalOutput"),
            "d_sel": nc.dram_tensor("d_sel", [128, 128], f32, kind="ExternalOutput"),
            "d_selT": nc.dram_tensor("d_selT", [128, 128], f32, kind="ExternalOutput"),
            "d_psb": nc.dram_tensor("d_psb", [128, cfg.mcap * cfg.layers[0][2]], f32, kind="ExternalOutput"),
            "d_ops": nc.dram_tensor("d_ops", [128, cfg.layers[0][1] + cfg.layers[0][2]], f32, kind="ExternalOutput"),
            "d_adst": nc.dram_tensor("d_adst", [128, cfg.layers[0][2]], f32, kind="ExternalOutput"),
            "d_adeps": nc.dram_tensor("d_adeps", [128, cfg.mcap * cfg.layers[0][2]], f32, kind="ExternalOutput"),
        }
    alpha_out = [nc.dram_tensor(f"alpha{li + 1}", [128, T * h], f32,
                                kind="ExternalOutput")
                 for li, (_, _, h) in enumerate(cfg.layers)]

    rg = [list(range(cfg.ncores))]

    with tile.TileContext(nc) as tc:
        with tc.tile_pool(name="const", bufs=1) as constp, \
             tc.tile_pool(name="resident", bufs=1) as resp, \
             tc.tile_pool(name="dram", bufs=1, space="DRAM") as dramp:

            # ---- constants
            identity = constp.tile([128, 128], f32, name="identity")
            make_identity(nc, identity[:])
            iota_row = constp.tile([128, 128], f32, name="iota_row")
            nc.gpsimd.iota(iota_row[:], pattern=[[1, 128]], base=0,
                           channel_multiplier=0,
                           allow_small_or_imprecise_dtypes=True)
            if dump:
                nc.sync.dma_start(dmp["d_iota"][:, :], iota_row[:])

            # ---- resident edge metadata
            gsrc_sb = resp.tile([128, T], i32, name="gsrc_sb")
            nc.sync.dma_start(gsrc_sb[:], gsrc[:, :])
            dstrel_sb = resp.tile([128, T], f32, name="dstrel_sb")
            nc.sync.dma_start(dstrel_sb[:], dstrel[:, :])

            # ---- big DRAM scratch
            xsl_d = dramp.tile([S * XM], f32, name="xsl_d")
            xfull_ds = [dramp.tile([NP * _xcols(fo_, h_)], f32,
                                   name=f"xfull_d{li_}", addr_space="Shared")
                        for li_, (_, fo_, h_) in enumerate(cfg.layers)]
            xT_d = dramp.tile([128, S], f32, name="xT_d")      # relu(x) transposed
            out_d = [dramp.tile([S, fo], f32, name=f"out_d{li}")
                     for li, (_, fo, _) in enumerate(cfg.layers)]

            for li, (fi, fo, h) in enumerate(cfg.layers):
                xc = _xcols(fo, h)
                glast = li == len(cfg.layers) - 1
                xsl_v = xsl_d[:].flatten()[:S * xc].rearrange("(n c) -> n c", c=xc)
                xfull_flat = xfull_ds[li][:].flatten()
                xfull_v = xfull_flat[:NP * xc].rearrange("(n c) -> n c", c=xc)

                # ---- bias broadcast tile
                with tc.tile_pool(name=f"bias{li}", bufs=1) as biasp:
                    bias_bc = biasp.tile([128, fo], f32, name=f"bias_bc{li}")
                    nc.sync.dma_start(bias_bc[:1, :], bias_d[li][:, :])
                    nc.gpsimd.partition_broadcast(bias_bc[:], bias_bc[:1, :])

                    # ================= node phase =================
                    with tc.tile_pool(name="np_w", bufs=1) as wp, \
                         tc.tile_pool(name="np_lhs", bufs=3) as lhsp, \
                         tc.tile_pool(name="np_ps", bufs=2, space="PSUM") as npp, \
                         tc.tile_pool(name="np_st", bufs=3) as stp:
                        wt = wp.tile([fi, xc], f32, name=f"wt{li}")
                        nc.sync.dma_start(wt[:], wext_d[li][:, :])
                        adst_res = resp.tile([128, B * h], f32,
                                             name=f"adst_res{li}")
                        for b in range(B):
                            lhs = lhsp.tile([fi, 128], f32, tag="lhs")
                            if li == 0:
                                nc.sync.dma_start(
                                    lhs[:], x0T[:, b * 128:(b + 1) * 128])
                            else:
                                nc.sync.dma_start(
                                    lhs[:], xT_d[:, b * 128:(b + 1) * 128])
                            ps = npp.tile([128, xc], f32, tag="nps")
                            nc.tensor.matmul(ps[:], lhsT=lhs[:], rhs=wt[:],
                                             start=True, stop=True)
                            st = stp.tile([128, xc], f32, tag="nst")
                            nc.scalar.copy(st[:], ps[:])
                            nc.vector.tensor_copy(
                                adst_res[:, b * h:(b + 1) * h],
                                ps[:, fo + h:fo + 2 * h])
                            nc.sync.dma_start(
                                xsl_v[b * 128:(b + 1) * 128, :], st[:])

                    # ================= allgather =================
                    if dump and li == 0:
                        nc.sync.dma_start(dmp["d_xsl"][:, :], xsl_v[:, :])
                    nc.gpsimd.collective_compute(
                        "AllGather", OP.bypass, replica_groups=rg,
                        ins=[xsl_d[:].flatten()[:S * xc]],
                        outs=[xfull_flat[:NP * xc]])

                    if dump and li == 0:
                        nc.sync.dma_start(dmp["d_xfull"][:, :], xfull_v[:, :])
                    # ================= edge phase =================
                    with tc.tile_pool(name="eg_g", bufs=3) as gp, \
                         tc.tile_pool(name="eg_sel", bufs=2) as selp, \
                         tc.tile_pool(name="eg_selT", bufs=2) as selTp, \
                         tc.tile_pool(name="eg_trps", bufs=2, space="PSUM") as trp, \
                         tc.tile_pool(name="eg_adps", bufs=2, space="PSUM") as adpsp, \
                         tc.tile_pool(name="eg_alps", bufs=2, space="PSUM") as alpsp, \
                         tc.tile_pool(name="eg_ops", bufs=2, space="PSUM") as opsp, \
                         tc.tile_pool(name="eg_sm", bufs=3) as smp, \
                         tc.tile_pool(name="eg_rhs", bufs=2) as rhsp, \
                         tc.tile_pool(name="eg_ep", bufs=2) as epp:
                        for b in range(B):
                            gb = gp.tile([128, M * xc], f32, tag="gbuf")
                            for t in range(M):
                                g = b * M + t
                                nc.gpsimd.indirect_dma_start(
                                    out=gb[:, t * xc:(t + 1) * xc],
                                    out_offset=None,
                                    in_=xfull_v,
                                    in_offset=bass.IndirectOffsetOnAxis(
                                        ap=gsrc_sb[:, g:g + 1], axis=0))
                            gb3 = gb[:].rearrange("p (t c) -> p t c", c=xc)
                            if dump and li == 0 and b == 0:
                                nc.sync.dma_start(dmp["d_gbuf"][:, :], gb[:])
                                nc.sync.dma_start(dmp["d_adst"][:, :],
                                                  adst_res[:, :h])

                            adstblk = adst_res[:, b * h:(b + 1) * h]
                            sel = selp.tile([128, M * 128], f32, tag="sel")
                            selT = selTp.tile([128, M * 128], f32, tag="selT")
                            ade_ps = adpsp.tile([128, M * h], f32, tag="adps")
                            for t in range(M):
                                g = b * M + t
                                sl = sel[:, t * 128:(t + 1) * 128]
                                nc.vector.tensor_scalar(
                                    sl, iota_row[:],
                                    dstrel_sb[:, g:g + 1], None,
                                    op0=OP.is_equal)
                                tp = trp.tile([128, 128], f32, tag="trps")
                                nc.tensor.transpose(tp[:], sl, identity[:])
                                sT = selT[:, t * 128:(t + 1) * 128]
                                if t % 2 == 0:
                                    nc.vector.tensor_copy(sT, tp[:])
                                else:
                                    nc.scalar.copy(sT, tp[:])
                                nc.tensor.matmul(
                                    ade_ps[:, t * h:(t + 1) * h],
                                    lhsT=sT, rhs=adstblk,
                                    start=True, stop=True)
                                if dump and li == 0 and b == 0 and t == 0:
                                    nc.sync.dma_start(dmp["d_sel"][:, :], sl)
                                    nc.sync.dma_start(dmp["d_selT"][:, :], sT)

                            # batched per-block softmax numerators
                            e_sb = smp.tile([128, M * h], f32, tag="esb")
                            nc.vector.tensor_tensor(
                                out=e_sb[:].rearrange("p (t h) -> p t h", h=h),
                                in0=ade_ps[:].rearrange("p (t h) -> p t h", h=h),
                                in1=gb3[:, :, fo:fo + h],
                                op=OP.add)
                            lr_sb = smp.tile([128, M * h], f32, tag="lrsb")
                            nc.vector.scalar_tensor_tensor(
                                out=lr_sb[:], in0=e_sb[:], scalar=NEG_SLOPE,
                                in1=e_sb[:], op0=OP.mult, op1=OP.max)
                            p_sb = smp.tile([128, M * h], f32, tag="psb")
                            nc.scalar.activation(p_sb[:], lr_sb[:], AT.Exp)
                            p3 = p_sb[:].rearrange("p (t h) -> p t h", h=h)
                            if dump and li == 0 and b == 0:
                                nc.sync.dma_start(dmp["d_psb"][:, :], p_sb[:])
                                nc.sync.dma_start(dmp["d_adeps"][:, :], e_sb[:])

                            # rhs = [p * xw | p]
                            rhs = rhsp.tile([128, M * (fo + h)], f32, tag="rhs")
                            rhs3 = rhs[:].rearrange("p (t c) -> p t c", c=fo + h)
                            c = fo // h
                            for hh in range(h):
                                nc.vector.tensor_tensor(
                                    out=rhs3[:, :, hh * c:(hh + 1) * c],
                                    in0=gb3[:, :, hh * c:(hh + 1) * c],
                                    in1=p3[:, :, hh:hh + 1].to_broadcast(
                                        [128, M, c]),
                                    op=OP.mult)
                            nc.vector.tensor_copy(rhs3[:, :, fo:fo + h], p3)

                            # main accumulation
                            ops = opsp.tile([128, fo + h], f32, tag="ops")
                            for t in range(M):
                                nc.tensor.matmul(
                                    ops[:],
                                    lhsT=sel[:, t * 128:(t + 1) * 128],
                                    rhs=rhs3[:, t, :],
                                    start=(t == 0), stop=(t == M - 1))

                            # epilogue: normalize, bias, store
                            den = epp.tile([128, h], f32, tag="den")
                            nc.vector.tensor_scalar(den[:], ops[:, fo:fo + h],
                                                    1e-16, None, op0=OP.add)
                            rden = epp.tile([128, h], f32, tag="rden")
                            nc.vector.reciprocal(rden[:], den[:])
                            xb = epp.tile([128, fo], f32, tag="xb")
                            nc.vector.tensor_tensor(
                                out=xb[:].rearrange("p (h c) -> p h c", h=h),
                                in0=ops[:, :fo].rearrange("p (h c) -> p h c", h=h),
                                in1=rden[:].unsqueeze(2).to_broadcast([128, h, c]),
                                op=OP.mult)
                            xbb = epp.tile([128, fo], f32, tag="xbb")
                            nc.vector.tensor_tensor(out=xbb[:], in0=xb[:],
                                                    in1=bias_bc[:], op=OP.add)
                            if dump and li == 0 and b == 0:
                                nc.sync.dma_start(
                                    dmp["d_ops"][:, :fo], xbb[:])
                            nc.sync.dma_start(
                                out_d[li][b * 128:(b + 1) * 128, :], xbb[:])
                            if not glast:
                                xr = epp.tile([128, fo], f32, tag="xr")
                                nc.scalar.activation(xr[:], xbb[:], AT.Relu)
                                xrt = trp.tile([128, 128], f32, tag="trps")
                                nc.tensor.transpose(xrt[:, :fo], xr[:],
                                                    identity[:])
                                xrs = epp.tile([128, 128], f32, tag="xrs")
                                nc.vector.tensor_copy(xrs[:fo, :], xrt[:fo, :])
                                nc.sync.dma_start(
                                    xT_d[:fo, b * 128:(b + 1) * 128],
                                    xrs[:fo, :])

                            # pass B: alpha = p * (selT @ rden)
                            al_ps = alpsp.tile([128, M * h], f32, tag="alps")
                            for t in range(M):
                                nc.tensor.matmul(
                                    al_ps[:, t * h:(t + 1) * h],
                                    lhsT=selT[:, t * 128:(t + 1) * 128],
                                    rhs=rden[:], start=True, stop=True)
                            al_sb = smp.tile([128, M * h], f32, tag="alsb")
                            nc.vector.tensor_tensor(out=al_sb[:], in0=al_ps[:],
                                                    in1=p_sb[:], op=OP.mult)
                            nc.sync.dma_start(
                                alpha_out[li][:, b * M * h:(b + 1) * M * h],
                                al_sb[:])

                            # final combine fused into last layer's epilogue
                            if glast:
                                f0 = epp.tile([128, 64], f32, tag="f0")
                                o1 = epp.tile([128, 128], f32, tag="o1")
                                o2 = epp.tile([128, 128], f32, tag="o2")
                                nc.sync.dma_start(
                                    f0[:], x0s[b * 128:(b + 1) * 128, :])
                                nc.sync.dma_start(
                                    o1[:], out_d[0][b * 128:(b + 1) * 128, :])
                                nc.sync.dma_start(
                                    o2[:], out_d[1][b * 128:(b + 1) * 128, :])
                                t1 = epp.tile([128, 64], f32, tag="t1")
                                nc.vector.tensor_tensor(
                                    out=t1[:], in0=o1[:, :64], in1=o1[:, 64:],
                                    op=OP.add)
                                t2 = epp.tile([128, 64], f32, tag="t2")
                                nc.vector.tensor_tensor(
                                    out=t2[:], in0=o2[:, :64], in1=o2[:, 64:],
                                    op=OP.add)
                                s1 = epp.tile([128, 64], f32, tag="s1")
                                nc.vector.scalar_tensor_tensor(
                                    out=s1[:], in0=t1[:], scalar=0.5,
                                    in1=f0[:], op0=OP.mult, op1=OP.add)
                                s2 = epp.tile([128, 64], f32, tag="s2")
                                nc.vector.scalar_tensor_tensor(
                                    out=s2[:], in0=t2[:], scalar=0.5,
                                    in1=s1[:], op0=OP.mult, op1=OP.add)
                                s3 = epp.tile([128, 64], f32, tag="s3")
                                nc.vector.tensor_tensor(
                                    out=s3[:], in0=s2[:], in1=xbb[:],
                                    op=OP.add)
                                fin = epp.tile([128, 64], f32, tag="fin")
                                nc.vector.tensor_scalar(
                                    fin[:], s3[:], 0.25, None, op0=OP.mult)
                                nc.sync.dma_start(
                                    final_out[b * 128:(b + 1) * 128, :],
                                    fin[:])

    nc.compile()
    return nc


# ---------------------------------------------------------------- entry

_CACHED = {}


def _run_on_hw(in_maps):
    sys.path.insert(0, "/opt/trn_rl_repo")
    from concourse import bass_utils
    if "nc" not in _CACHED:
        _CACHED["nc"] = build_nc()
    nc = _CACHED["nc"]
    kw = {}
    if os.environ.get("GAT_TRACE"):
        sys.path.insert(0, "/root/problem")
        import axon_prof
        axon_prof.install()
        kw = dict(trace=True, tmpdir=os.environ.get("GAT_TRACE"),
                  trace_cores=[int(os.environ.get("GAT_TRACE_CORE", "0"))])
    res = bass_utils.run_bass_kernel_spmd(
        nc, in_maps, core_ids=list(range(NCORES)), **kw)
    if res.exec_time_ns is not None:
        print(f"HW exec time: {res.exec_time_ns} ns")
    return res.results


def kernel(**inputs):
    inputs = {k: np.asarray(v) for k, v in inputs.items()}
    in_maps, origs = host_prep_all(inputs)
    results = _run_on_hw(in_maps)
    return postprocess(results, origs)


# revision 14
# speedup vs baseline: 1.1723x; 1.1723x over previous
"""Trainium2 Bass kernel for the 3-layer GAT (nn_GAT_56118042689980).

Strategy: destination-node sharding across 8 cores.
  - Host (numpy, data layout only): build W_ext = [W | W@att_src | W@att_dst],
    sort edges by dst, partition by dst range, group into 128-dst blocks x
    128-edge tiles, pad uniformly.
  - Per layer on chip: node phase (sliced matmul x@W_ext -> AllGather full
    table), edge phase (indirect-DMA row gather by src, one-hot sel matmuls
    accumulating weighted sums + softmax denominators in PSUM, selT matmuls
    broadcasting per-dst values back to edges).
"""

import os
import sys
from dataclasses import dataclass

import ml_dtypes
import numpy as np

# ---------------------------------------------------------------- constants

I_ITEMS, U_USERS, F_FETS = 40000, 30000, 30000
N_NODES = 100000
D_EMB = 64
E_EDGES = 1200000
NEG_SLOPE = 0.2
NCORES = 8

SLICE = 12544            # nodes per core; NPAD = 8*12544 = 100352 >= 100000
NPAD = SLICE * NCORES
BLOCKS = 98              # dst blocks of 128 per core (98*128 = 12544)
MCAP = 15                # edge tiles (of 128) per block, uniform padding
TTOT = BLOCKS * MCAP     # edge tiles per core
EPAD = TTOT * 128        # edge slots per core
PAD_NODE = NPAD - 1      # gather index for pad edge slots (a zero row)
PAD_DSTREL = 999.0       # sentinel; is_equal never matches -> zero sel column

# per layer: (F_in, F_out, heads)
LAYERS = [(64, 128, 2), (128, 128, 2), (128, 64, 1)]
XCOLS_MAX = 132          # F_out + 2*H max


def _xcols(fo, h):
    return fo + 2 * h


# ---------------------------------------------------------------- host prep

def build_w_ext(W, att_src, att_dst):
    """W_ext = [W | W @ att_src per head | W @ att_dst per head]  (f32)."""
    fin, fohh = W.shape
    h, c = att_src.shape
    fo = h * c
    assert fohh == fo
    out = np.zeros((fin, fo + 2 * h), np.float32)
    out[:, :fo] = W
    for hh in range(h):
        out[:, fo + hh] = W[:, hh * c:(hh + 1) * c] @ att_src[hh]
        out[:, fo + h + hh] = W[:, hh * c:(hh + 1) * c] @ att_dst[hh]
    return out


def host_prep_edges(edge_index, n_nodes=N_NODES, slice_=SLICE, blocks=BLOCKS,
                    mcap=MCAP, ncores=NCORES):
    """Sort edges (incl. self loops) by dst, partition by dst range, lay out
    into per-core [128, TTOT] slot grids.

    Returns per-core dicts with gsrc (int32 gather idx), dstrel (f32),
    orig (int64 original edge id, -1 for pads)."""
    npad = slice_ * ncores
    ttot = blocks * mcap
    pad_node = npad - 1
    src = np.concatenate([edge_index[0], np.arange(n_nodes, dtype=np.int32)])
    dst = np.concatenate([edge_index[1], np.arange(n_nodes, dtype=np.int32)])
    order = np.argsort(dst, kind="stable")
    src_s = src[order].astype(np.int64)
    dst_s = dst[order].astype(np.int64)

    cores = []
    bounds = np.searchsorted(dst_s, np.arange(ncores + 1) * slice_)
    for k in range(ncores):
        a, b = bounds[k], bounds[k + 1]
        dl = dst_s[a:b] - k * slice_
        blk = dl >> 7
        rel = dl & 127
        # rank of each edge within its block (edges are dst-sorted => block-grouped)
        blk_start = np.searchsorted(blk, np.arange(blocks))
        r = np.arange(b - a) - blk_start[blk]
        assert r.max(initial=0) < mcap * 128, (
            f"core {k}: block overflow {r.max()} >= {mcap * 128}")
        t = r >> 7
        p = r & 127
        g = blk * mcap + t
        gsrc = np.full((128, ttot), pad_node, np.int32)
        dstrel = np.full((128, ttot), PAD_DSTREL, np.float32)
        orig = np.full((128, ttot), -1, np.int64)
        gsrc[p, g] = src_s[a:b]
        dstrel[p, g] = rel
        orig[p, g] = order[a:b]
        cores.append(dict(gsrc=gsrc, dstrel=dstrel, orig=orig))
    return cores


def host_prep_all(inputs, cfg=None, n_real=N_NODES):
    """All host-side arrays: per-core input maps + postprocess info."""
    cfg = cfg or Cfg()
    x0 = np.concatenate(
        [inputs["emb_item"], inputs["emb_user"], inputs["emb_fet"]], axis=0)
    d_emb = x0.shape[1]
    x0p = np.zeros((cfg.npad, d_emb), np.float32)
    x0p[:n_real] = x0

    wexts = [
        build_w_ext(inputs["W1"], inputs["att_src1"], inputs["att_dst1"]),
        build_w_ext(inputs["W2"], inputs["att_src2"], inputs["att_dst2"]),
        build_w_ext(inputs["W3"], inputs["att_src3"], inputs["att_dst3"]),
    ]
    biases = [np.asarray(inputs["b1"], np.float32).reshape(1, -1),
              np.asarray(inputs["b2"], np.float32).reshape(1, -1),
              np.asarray(inputs["b3"], np.float32).reshape(1, -1)]

    edge_cores = host_prep_edges(inputs["edge_index"], n_nodes=n_real,
                                 slice_=cfg.slice_, blocks=cfg.blocks,
                                 mcap=cfg.mcap, ncores=cfg.ncores)

    in_maps = []
    for k in range(cfg.ncores):
        sl = x0p[k * cfg.slice_:(k + 1) * cfg.slice_]
        m = {
            "x0T": np.ascontiguousarray(sl.T),        # [64, SLICE]
            "x0s": np.ascontiguousarray(sl),          # [SLICE, 64]
            "gsrc": edge_cores[k]["gsrc"],            # [128, TTOT] int32
            "dstrel": edge_cores[k]["dstrel"],        # [128, TTOT] f32
            "Wext1": wexts[0], "Wext2": wexts[1], "Wext3": wexts[2],
            "bias1": biases[0], "bias2": biases[1], "bias3": biases[2],
        }
        in_maps.append(m)
    origs = [edge_cores[k]["orig"] for k in range(cfg.ncores)]
    return in_maps, origs


def postprocess(results, origs, cfg=None, n_real=N_NODES, e_real=E_EDGES,
                splits=(I_ITEMS, U_USERS)):
    """Assemble full outputs from per-core results."""
    cfg = cfg or Cfg()
    final = np.concatenate([r["final_out"] for r in results], axis=0)[:n_real]
    ep = e_real + n_real
    alphas = []
    for li, (_, _, h) in enumerate(cfg.layers):
        full = np.zeros((ep, h), np.float32)
        for k in range(cfg.ncores):
            v = results[k][f"alpha{li + 1}"].reshape(128, cfg.ttot, h)
            o = origs[k]
            msk = o >= 0
            full[o[msk]] = v[msk]
        alphas.append(full)
    a, b = splits
    return (final[:a], final[a:a + b],
            final[a + b:], final, alphas[0], alphas[1], alphas[2])


# ---------------------------------------------------------------- bass build

@dataclass
class Cfg:
    slice_: int = SLICE
    blocks: int = BLOCKS
    mcap: int = MCAP
    ncores: int = NCORES
    layers: tuple = ((64, 128, 2), (128, 128, 2), (128, 64, 1))

    @property
    def npad(self):
        return self.slice_ * self.ncores

    @property
    def ttot(self):
        return self.blocks * self.mcap

    @property
    def xcols_max(self):
        return max(_xcols(fo, h) for _, fo, h in self.layers)


def build_nc(cfg: Cfg = Cfg(), debug=False, dump=False):
    import concourse.bass as bass
    import concourse.mybir as mybir
    import concourse.tile as tile
    from concourse import bacc
    from concourse.masks import make_identity

    f32 = mybir.dt.float32
    bf16 = mybir.dt.bfloat16
    i32 = mybir.dt.int32
    AT = mybir.ActivationFunctionType
    OP = mybir.AluOpType

    S, B, M, T = cfg.slice_, cfg.blocks, cfg.mcap, cfg.ttot
    NP = cfg.npad
    XM = cfg.xcols_max
    fin1 = cfg.layers[0][0]

    nc = bacc.Bacc("TRN2", target_bir_lowering=False, debug=debug,
                   enable_asserts=False, num_devices=cfg.ncores)

    # ---- I/O
    x0T = nc.dram_tensor("x0T", [fin1, S], f32, kind="ExternalInput")
    x0s = nc.dram_tensor("x0s", [S, fin1], f32, kind="ExternalInput")
    gsrc = nc.dram_tensor("gsrc", [128, T], i32, kind="ExternalInput")
    dstrel = nc.dram_tensor("dstrel", [128, T], f32, kind="ExternalInput")
    wext_d, bias_d = [], []
    for li, (fi, fo, h) in enumerate(cfg.layers):
        wext_d.append(nc.dram_tensor(f"Wext{li + 1}", [fi, _xcols(fo, h)], f32,
                                     kind="ExternalInput"))
        bias_d.append(nc.dram_tensor(f"bias{li + 1}", [1, fo], f32,
                                     kind="ExternalInput"))
    final_out = nc.dram_tensor("final_out", [S, 64], f32, kind="ExternalOutput")
    if dump:
        dmp = {
            "d_iota": nc.dram_tensor("d_iota", [128, 128], f32, kind="ExternalOutput"),
            "d_xsl": nc.dram_tensor("d_xsl", [S, _xcols(cfg.layers[0][1], cfg.layers[0][2])], f32, kind="ExternalOutput"),
            "d_xfull": nc.dram_tensor("d_xfull", [NP, _xcols(cfg.layers[0][1], cfg.layers[0][2])], f32, kind="ExternalOutput"),
            "d_gbuf": nc.dram_tensor("d_gbuf", [128, cfg.mcap * _xcols(cfg.layers[0][1], cfg.layers[0][2])], f32, kind="ExternalOutput"),
            "d_sel": nc.dram_tensor("d_sel", [128, 128], f32, kind="ExternalOutput"),
            "d_selT": nc.dram_tensor("d_selT", [128, 128], f32, kind="ExternalOutput"),
            "d_psb": nc.dram_tensor("d_psb", [128, cfg.mcap * cfg.layers[0][2]], f32, kind="ExternalOutput"),
            "d_ops": nc.dram_tensor("d_ops", [128, cfg.layers[0][1] + cfg.layers[0][2]], f32, kind="ExternalOutput"),
            "d_adst": nc.dram_tensor("d_adst", [128, cfg.layers[0][2]], f32, kind="ExternalOutput"),
            "d_adeps": nc.dram_tensor("d_adeps", [128, cfg.mcap * cfg.layers[0][2]], f32, kind="ExternalOutput"),
        }
    alpha_out = [nc.dram_tensor(f"alpha{li + 1}", [128, T * h], f32,
                                kind="ExternalOutput")
                 for li, (_, _, h) in enumerate(cfg.layers)]

    rg = [list(range(cfg.ncores))]

    with tile.TileContext(nc) as tc:
        with tc.tile_pool(name="const", bufs=1) as constp, \
             tc.tile_pool(name="resident", bufs=1) as resp, \
             tc.tile_pool(name="dram", bufs=1, space="DRAM") as dramp:

            # ---- constants
            identity = constp.tile([128, 128], bf16, name="identity")
            make_identity(nc, identity[:])
            identity_f = constp.tile([128, 128], f32, name="identity_f")
            make_identity(nc, identity_f[:])
            iota_i32 = constp.tile([128, 128], i32, name="iota_i32")
            nc.gpsimd.iota(iota_i32[:], pattern=[[1, 128]], base=0,
                           channel_multiplier=0)
            iota_row = constp.tile([128, 128], bf16, name="iota_row")
            nc.vector.tensor_copy(iota_row[:], iota_i32[:])
            if dump:
                nc.sync.dma_start(dmp["d_iota"][:, :], iota_row[:])

            # ---- resident edge metadata
            gsrc_sb = resp.tile([128, T], i32, name="gsrc_sb")
            nc.sync.dma_start(gsrc_sb[:], gsrc[:, :])
            dstrel_sb = resp.tile([128, T], f32, name="dstrel_sb")
            nc.sync.dma_start(dstrel_sb[:], dstrel[:, :])

            # ---- big DRAM scratch
            xsl_d = dramp.tile([S * XM], f32, name="xsl_d")
            xfull_ds = [dramp.tile([NP * _xcols(fo_, h_)], f32,
                                   name=f"xfull_d{li_}", addr_space="Shared")
                        for li_, (_, fo_, h_) in enumerate(cfg.layers)]
            xT_d = dramp.tile([128, S], f32, name="xT_d")      # relu(x) transposed
            out_d = [dramp.tile([S, fo], f32, name=f"out_d{li}")
                     for li, (_, fo, _) in enumerate(cfg.layers)]

            for li, (fi, fo, h) in enumerate(cfg.layers):
                xc = _xcols(fo, h)
                glast = li == len(cfg.layers) - 1
                xsl_v = xsl_d[:].flatten()[:S * xc].rearrange("(n c) -> n c", c=xc)
                xfull_flat = xfull_ds[li][:].flatten()
                xfull_v = xfull_flat[:NP * xc].rearrange("(n c) -> n c", c=xc)

                # ---- bias broadcast tile
                with tc.tile_pool(name=f"bias{li}", bufs=1) as biasp:
                    bias_bc = biasp.tile([128, fo], f32, name=f"bias_bc{li}")
                    nc.sync.dma_start(bias_bc[:1, :], bias_d[li][:, :])
                    nc.gpsimd.partition_broadcast(bias_bc[:], bias_bc[:1, :])

                    # ================= node phase =================
                    with tc.tile_pool(name="np_w", bufs=1) as wp, \
                         tc.tile_pool(name="np_lhs", bufs=3) as lhsp, \
                         tc.tile_pool(name="np_ps", bufs=2, space="PSUM") as npp, \
                         tc.tile_pool(name="np_st", bufs=3) as stp:
                        wt = wp.tile([fi, xc], f32, name=f"wt{li}")
                        nc.sync.dma_start(wt[:], wext_d[li][:, :])
                        adst_res = resp.tile([128, B * h], bf16,
                                             name=f"adst_res{li}")
                        for b in range(B):
                            lhs = lhsp.tile([fi, 128], f32, tag="lhs")
                            if li == 0:
                                nc.sync.dma_start(
                                    lhs[:], x0T[:, b * 128:(b + 1) * 128])
                            else:
                                nc.sync.dma_start(
                                    lhs[:], xT_d[:, b * 128:(b + 1) * 128])
                            ps = npp.tile([128, xc], f32, tag="nps")
                            nc.tensor.matmul(ps[:], lhsT=lhs[:], rhs=wt[:],
                                             start=True, stop=True)
                            st = stp.tile([128, xc], f32, tag="nst")
                            nc.scalar.copy(st[:], ps[:])
                            nc.vector.tensor_copy(
                                adst_res[:, b * h:(b + 1) * h],
                                ps[:, fo + h:fo + 2 * h])
                            nc.sync.dma_start(
                                xsl_v[b * 128:(b + 1) * 128, :], st[:])

                    # ================= allgather =================
                    if dump and li == 0:
                        nc.sync.dma_start(dmp["d_xsl"][:, :], xsl_v[:, :])
                    nc.gpsimd.collective_compute(
                        "AllGather", OP.bypass, replica_groups=rg,
                        ins=[xsl_d[:].flatten()[:S * xc]],
                        outs=[xfull_flat[:NP * xc]])

                    if dump and li == 0:
                        nc.sync.dma_start(dmp["d_xfull"][:, :], xfull_v[:, :])
                    # ================= edge phase =================
                    with tc.tile_pool(name="eg_g", bufs=3) as gp, \
                         tc.tile_pool(name="eg_sel", bufs=2) as selp, \
                         tc.tile_pool(name="eg_selT", bufs=2) as selTp, \
                         tc.tile_pool(name="eg_trps", bufs=2, space="PSUM") as trp, \
                         tc.tile_pool(name="eg_adps", bufs=2, space="PSUM") as adpsp, \
                         tc.tile_pool(name="eg_alps", bufs=2, space="PSUM") as alpsp, \
                         tc.tile_pool(name="eg_ops", bufs=2, space="PSUM") as opsp, \
                         tc.tile_pool(name="eg_sm", bufs=3) as smp, \
                         tc.tile_pool(name="eg_rhs", bufs=2) as rhsp, \
                         tc.tile_pool(name="eg_ep", bufs=2) as epp:
                        for b in range(B):
                            gb = gp.tile([128, M * xc], f32, tag="gbuf")
                            for t in range(M):
                                g = b * M + t
                                nc.gpsimd.indirect_dma_start(
                                    out=gb[:, t * xc:(t + 1) * xc],
                                    out_offset=None,
                                    in_=xfull_v,
                                    in_offset=bass.IndirectOffsetOnAxis(
                                        ap=gsrc_sb[:, g:g + 1], axis=0))
                            gb3 = gb[:].rearrange("p (t c) -> p t c", c=xc)
                            if dump and li == 0 and b == 0:
                                nc.sync.dma_start(dmp["d_gbuf"][:, :], gb[:])
                                nc.sync.dma_start(dmp["d_adst"][:, :],
                                                  adst_res[:, :h])

                            adstblk = adst_res[:, b * h:(b + 1) * h]
                            sel = selp.tile([128, M * 128], bf16, tag="sel")
                            selT = selTp.tile([128, M * 128], bf16, tag="selT")
                            ade_ps = adpsp.tile([128, M * h], f32, tag="adps")
                            for t in range(M):
                                g = b * M + t
                                sl = sel[:, t * 128:(t + 1) * 128]
                                nc.vector.tensor_scalar(
                                    sl, iota_row[:],
                                    dstrel_sb[:, g:g + 1], None,
                                    op0=OP.is_equal)
                                tp = trp.tile([128, 128], bf16, tag="trps")
                                nc.tensor.transpose(tp[:], sl, identity[:])
                                sT = selT[:, t * 128:(t + 1) * 128]
                                nc.scalar.copy(sT, tp[:])
                                nc.tensor.matmul(
                                    ade_ps[:, t * h:(t + 1) * h],
                                    lhsT=sT, rhs=adstblk,
                                    start=True, stop=True)
                                if dump and li == 0 and b == 0 and t == 0:
                                    nc.sync.dma_start(dmp["d_sel"][:, :], sl)
                                    nc.sync.dma_start(dmp["d_selT"][:, :], sT)

                            # batched per-block softmax numerators
                            e_sb = smp.tile([128, M * h], f32, tag="esb")
                            nc.vector.tensor_tensor(
                                out=e_sb[:].rearrange("p (t h) -> p t h", h=h),
                                in0=ade_ps[:].rearrange("p (t h) -> p t h", h=h),
                                in1=gb3[:, :, fo:fo + h],
                                op=OP.add)
                            lr_sb = smp.tile([128, M * h], f32, tag="lrsb")
                            nc.vector.scalar_tensor_tensor(
                                out=lr_sb[:], in0=e_sb[:], scalar=NEG_SLOPE,
                                in1=e_sb[:], op0=OP.mult, op1=OP.max)
                            p_sb = smp.tile([128, M * h], f32, tag="psb")
                            nc.scalar.activation(p_sb[:], lr_sb[:], AT.Exp)
                            p3 = p_sb[:].rearrange("p (t h) -> p t h", h=h)
                            if dump and li == 0 and b == 0:
                                nc.sync.dma_start(dmp["d_psb"][:, :], p_sb[:])
                                nc.sync.dma_start(dmp["d_adeps"][:, :], e_sb[:])

                            # rhs = [p * xw | p]
                            rhs = rhsp.tile([128, M * (fo + h)], bf16, tag="rhs")
                            rhs3 = rhs[:].rearrange("p (t c) -> p t c", c=fo + h)
                            c = fo // h
                            for hh in range(h):
                                nc.vector.tensor_tensor(
                                    out=rhs3[:, :, hh * c:(hh + 1) * c],
                                    in0=gb3[:, :, hh * c:(hh + 1) * c],
                                    in1=p3[:, :, hh:hh + 1].to_broadcast(
                                        [128, M, c]),
                                    op=OP.mult)
                            nc.vector.tensor_copy(rhs3[:, :, fo:fo + h], p3)

                            # main accumulation
                            ops = opsp.tile([128, fo + h], f32, tag="ops")
                            for t in range(M):
                                nc.tensor.matmul(
                                    ops[:],
                                    lhsT=sel[:, t * 128:(t + 1) * 128],
                                    rhs=rhs3[:, t, :],
                                    start=(t == 0), stop=(t == M - 1))

                            # epilogue: normalize, bias, store
                            den = epp.tile([128, h], f32, tag="den")
                            nc.vector.tensor_scalar(den[:], ops[:, fo:fo + h],
                                                    1e-16, None, op0=OP.add)
                            rden = epp.tile([128, h], f32, tag="rden")
                            nc.vector.reciprocal(rden[:], den[:])
                            rdenb = epp.tile([128, h], bf16, tag="rdenb")
                            nc.vector.tensor_copy(rdenb[:], rden[:])
                            xb = epp.tile([128, fo], f32, tag="xb")
                            nc.vector.tensor_tensor(
                                out=xb[:].rearrange("p (h c) -> p h c", h=h),
                                in0=ops[:, :fo].rearrange("p (h c) -> p h c", h=h),
                                in1=rden[:].unsqueeze(2).to_broadcast([128, h, c]),
                                op=OP.mult)
                            xbb = epp.tile([128, fo], f32, tag="xbb")
                            nc.vector.tensor_tensor(out=xbb[:], in0=xb[:],
                                                    in1=bias_bc[:], op=OP.add)
                            if dump and li == 0 and b == 0:
                                nc.sync.dma_start(
                                    dmp["d_ops"][:, :fo], xbb[:])
                            nc.sync.dma_start(
                                out_d[li][b * 128:(b + 1) * 128, :], xbb[:])
                            if not glast:
                                xr = epp.tile([128, fo], f32, tag="xr")
                                nc.scalar.activation(xr[:], xbb[:], AT.Relu)
                                xrt = trp.tile([128, 128], f32, tag="trps")
                                nc.tensor.transpose(xrt[:, :fo], xr[:],
                                                    identity_f[:])
                                xrs = epp.tile([128, 128], f32, tag="xrs")
                                nc.vector.tensor_copy(xrs[:fo, :], xrt[:fo, :])
                                nc.sync.dma_start(
                                    xT_d[:fo, b * 128:(b + 1) * 128],
                                    xrs[:fo, :])

                            # pass B: alpha = p * (selT @ rden)
                            al_ps = alpsp.tile([128, M * h], f32, tag="alps")
                            for t in range(M):
                                nc.tensor.matmul(
                                    al_ps[:, t * h:(t + 1) * h],
                                    lhsT=selT[:, t * 128:(t + 1) * 128],
                                    rhs=rdenb[:], start=True, stop=True)
                            al_sb = smp.tile([128, M * h], f32, tag="alsb")
                            nc.vector.tensor_tensor(out=al_sb[:], in0=al_ps[:],
                                                    in1=p_sb[:], op=OP.mult)
                            nc.sync.dma_start(
                                alpha_out[li][:, b * M * h:(b + 1) * M * h],
                                al_sb[:])

                            # final combine fused into last layer's epilogue
                            if glast:
                                f0 = epp.tile([128, 64], f32, tag="f0")
                                o1 = epp.tile([128, 128], f32, tag="o1")
                                o2 = epp.tile([128, 128], f32, tag="o2")
                                nc.sync.dma_start(
                                    f0[:], x0s[b * 128:(b + 1) * 128, :])
                                nc.sync.dma_start(
                                    o1[:], out_d[0][b * 128:(b + 1) * 128, :])
                                nc.sync.dma_start(
                                    o2[:], out_d[1][b * 128:(b + 1) * 128, :])
                                t1 = epp.tile([128, 64], f32, tag="t1")
                                nc.vector.tensor_tensor(
                                    out=t1[:], in0=o1[:, :64], in1=o1[:, 64:],
                                    op=OP.add)
                                t2 = epp.tile([128, 64], f32, tag="t2")
                                nc.vector.tensor_tensor(
                                    out=t2[:], in0=o2[:, :64], in1=o2[:, 64:],
                                    op=OP.add)
                                s1 = epp.tile([128, 64], f32, tag="s1")
                                nc.vector.scalar_tensor_tensor(
                                    out=s1[:], in0=t1[:], scalar=0.5,
                                    in1=f0[:], op0=OP.mult, op1=OP.add)
                                s2 = epp.tile([128, 64], f32, tag="s2")
                                nc.vector.scalar_tensor_tensor(
                                    out=s2[:], in0=t2[:], scalar=0.5,
                                    in1=s1[:], op0=OP.mult, op1=OP.add)
                                s3 = epp.tile([128, 64], f32, tag="s3")
                                nc.vector.tensor_tensor(
                                    out=s3[:], in0=s2[:], in1=xbb[:],
                                    op=OP.add)
                                fin = epp.tile([128, 64], f32, tag="fin")
                                nc.vector.tensor_scalar(
                                    fin[:], s3[:], 0.25, None, op0=OP.mult)
                                nc.sync.dma_start(
                                    final_out[b * 128:(b + 1) * 128, :],
                                    fin[:])

    nc.compile()
    return nc


# ---------------------------------------------------------------- entry

_CACHED = {}


def _run_on_hw(in_maps):
    sys.path.insert(0, "/opt/trn_rl_repo")
    from concourse import bass_utils
    if "nc" not in _CACHED:
        _CACHED["nc"] = build_nc()
    nc = _CACHED["nc"]
    kw = {}
    if os.environ.get("GAT_TRACE"):
        sys.path.insert(0, "/root/problem")
        import axon_prof
        axon_prof.install()
        kw = dict(trace=True, tmpdir=os.environ.get("GAT_TRACE"),
                  trace_cores=[int(os.environ.get("GAT_TRACE_CORE", "0"))])
    res = bass_utils.run_bass_kernel_spmd(
        nc, in_maps, core_ids=list(range(NCORES)), **kw)
    if res.exec_time_ns is not None:
        print(f"HW exec time: {res.exec_time_ns} ns")
    return res.results


def kernel(**inputs):
    inputs = {k: np.asarray(v) for k, v in inputs.items()}
    in_maps, origs = host_prep_all(inputs)
    results = _run_on_hw(in_maps)
    return postprocess(results, origs)


# revision 15
# speedup vs baseline: 1.2710x; 1.0842x over previous
"""Trainium2 Bass kernel for the 3-layer GAT (nn_GAT_56118042689980).

Strategy: destination-node sharding across 8 cores.
  - Host (numpy, data layout only): build W_ext = [W | W@att_src | W@att_dst],
    sort edges by dst, partition by dst range, group into 128-dst blocks x
    128-edge tiles, pad uniformly.
  - Per layer on chip: node phase (sliced matmul x@W_ext -> AllGather full
    table), edge phase (indirect-DMA row gather by src, one-hot sel matmuls
    accumulating weighted sums + softmax denominators in PSUM, selT matmuls
    broadcasting per-dst values back to edges).
"""

import os
import sys
from dataclasses import dataclass

import ml_dtypes
import numpy as np

# ---------------------------------------------------------------- constants

I_ITEMS, U_USERS, F_FETS = 40000, 30000, 30000
N_NODES = 100000
D_EMB = 64
E_EDGES = 1200000
NEG_SLOPE = 0.2
NCORES = 8

SLICE = 12544            # nodes per core; NPAD = 8*12544 = 100352 >= 100000
NPAD = SLICE * NCORES
BLOCKS = 98              # dst blocks of 128 per core (98*128 = 12544)
MCAP = 14                # edge tiles (of 128) per block, uniform padding
TTOT = BLOCKS * MCAP     # edge tiles per core
EPAD = TTOT * 128        # edge slots per core
PAD_NODE = NPAD - 1      # gather index for pad edge slots (a zero row)
PAD_DSTREL = 999.0       # sentinel; is_equal never matches -> zero sel column

# per layer: (F_in, F_out, heads)
LAYERS = [(64, 128, 2), (128, 128, 2), (128, 64, 1)]
XCOLS_MAX = 132          # F_out + 2*H max


def _xcols(fo, h):
    return fo + 2 * h


# ---------------------------------------------------------------- host prep

def build_w_ext(W, att_src, att_dst):
    """W_ext = [W | W @ att_src per head | W @ att_dst per head]  (f32)."""
    fin, fohh = W.shape
    h, c = att_src.shape
    fo = h * c
    assert fohh == fo
    out = np.zeros((fin, fo + 2 * h), np.float32)
    out[:, :fo] = W
    for hh in range(h):
        out[:, fo + hh] = W[:, hh * c:(hh + 1) * c] @ att_src[hh]
        out[:, fo + h + hh] = W[:, hh * c:(hh + 1) * c] @ att_dst[hh]
    return out


def host_prep_edges(edge_index, n_nodes=N_NODES, slice_=SLICE, blocks=BLOCKS,
                    mcap=MCAP, ncores=NCORES):
    """Sort edges (incl. self loops) by dst, partition by dst range, lay out
    into per-core [128, TTOT] slot grids.

    Returns per-core dicts with gsrc (int32 gather idx), dstrel (f32),
    orig (int64 original edge id, -1 for pads)."""
    npad = slice_ * ncores
    ttot = blocks * mcap
    pad_node = npad - 1
    src = np.concatenate([edge_index[0], np.arange(n_nodes, dtype=np.int32)])
    dst = np.concatenate([edge_index[1], np.arange(n_nodes, dtype=np.int32)])
    order = np.argsort(dst, kind="stable")
    src_s = src[order].astype(np.int64)
    dst_s = dst[order].astype(np.int64)

    cores = []
    bounds = np.searchsorted(dst_s, np.arange(ncores + 1) * slice_)
    for k in range(ncores):
        a, b = bounds[k], bounds[k + 1]
        dl = dst_s[a:b] - k * slice_
        blk = dl >> 7
        rel = dl & 127
        # rank of each edge within its block (edges are dst-sorted => block-grouped)
        blk_start = np.searchsorted(blk, np.arange(blocks))
        r = np.arange(b - a) - blk_start[blk]
        assert r.max(initial=0) < mcap * 128, (
            f"core {k}: block overflow {r.max()} >= {mcap * 128}")
        t = r >> 7
        p = r & 127
        g = blk * mcap + t
        gsrc = np.full((128, ttot), pad_node, np.int32)
        dstrel = np.full((128, ttot), PAD_DSTREL, np.float32)
        orig = np.full((128, ttot), -1, np.int64)
        gsrc[p, g] = src_s[a:b]
        dstrel[p, g] = rel
        orig[p, g] = order[a:b]
        cores.append(dict(gsrc=gsrc, dstrel=dstrel, orig=orig))
    return cores


def host_prep_all(inputs, cfg=None, n_real=N_NODES):
    """All host-side arrays: per-core input maps + postprocess info."""
    cfg = cfg or Cfg()
    x0 = np.concatenate(
        [inputs["emb_item"], inputs["emb_user"], inputs["emb_fet"]], axis=0)
    d_emb = x0.shape[1]
    x0p = np.zeros((cfg.npad, d_emb), np.float32)
    x0p[:n_real] = x0

    wexts = [
        build_w_ext(inputs["W1"], inputs["att_src1"], inputs["att_dst1"]),
        build_w_ext(inputs["W2"], inputs["att_src2"], inputs["att_dst2"]),
        build_w_ext(inputs["W3"], inputs["att_src3"], inputs["att_dst3"]),
    ]
    biases = [np.asarray(inputs["b1"], np.float32).reshape(1, -1),
              np.asarray(inputs["b2"], np.float32).reshape(1, -1),
              np.asarray(inputs["b3"], np.float32).reshape(1, -1)]

    edge_cores = host_prep_edges(inputs["edge_index"], n_nodes=n_real,
                                 slice_=cfg.slice_, blocks=cfg.blocks,
                                 mcap=cfg.mcap, ncores=cfg.ncores)

    in_maps = []
    for k in range(cfg.ncores):
        sl = x0p[k * cfg.slice_:(k + 1) * cfg.slice_]
        m = {
            "x0T": np.ascontiguousarray(sl.T),        # [64, SLICE]
            "x0s": np.ascontiguousarray(sl),          # [SLICE, 64]
            "gsrc": edge_cores[k]["gsrc"],            # [128, TTOT] int32
            "dstrel": edge_cores[k]["dstrel"].astype(ml_dtypes.bfloat16),
            "Wext1": wexts[0], "Wext2": wexts[1], "Wext3": wexts[2],
            "bias1": biases[0], "bias2": biases[1], "bias3": biases[2],
        }
        in_maps.append(m)
    origs = [edge_cores[k]["orig"] for k in range(cfg.ncores)]
    return in_maps, origs


def postprocess(results, origs, cfg=None, n_real=N_NODES, e_real=E_EDGES,
                splits=(I_ITEMS, U_USERS)):
    """Assemble full outputs from per-core results."""
    cfg = cfg or Cfg()
    final = np.concatenate([r["final_out"] for r in results], axis=0)[:n_real]
    ep = e_real + n_real
    alphas = []
    for li, (_, _, h) in enumerate(cfg.layers):
        full = np.zeros((ep, h), np.float32)
        for k in range(cfg.ncores):
            v = results[k][f"alpha{li + 1}"].reshape(128, cfg.ttot, h)
            o = origs[k]
            msk = o >= 0
            full[o[msk]] = v[msk]
        alphas.append(full)
    a, b = splits
    return (final[:a], final[a:a + b],
            final[a + b:], final, alphas[0], alphas[1], alphas[2])


# ---------------------------------------------------------------- bass build

@dataclass
class Cfg:
    slice_: int = SLICE
    blocks: int = BLOCKS
    mcap: int = MCAP
    ncores: int = NCORES
    layers: tuple = ((64, 128, 2), (128, 128, 2), (128, 64, 1))

    @property
    def npad(self):
        return self.slice_ * self.ncores

    @property
    def ttot(self):
        return self.blocks * self.mcap

    @property
    def xcols_max(self):
        return max(_xcols(fo, h) for _, fo, h in self.layers)


def build_nc(cfg: Cfg = Cfg(), debug=False, dump=False):
    import concourse.bass as bass
    import concourse.mybir as mybir
    import concourse.tile as tile
    from concourse import bacc
    from concourse.masks import make_identity

    f32 = mybir.dt.float32
    bf16 = mybir.dt.bfloat16
    i32 = mybir.dt.int32
    AT = mybir.ActivationFunctionType
    OP = mybir.AluOpType

    S, B, M, T = cfg.slice_, cfg.blocks, cfg.mcap, cfg.ttot
    NP = cfg.npad
    XM = cfg.xcols_max
    fin1 = cfg.layers[0][0]

    nc = bacc.Bacc("TRN2", target_bir_lowering=False, debug=debug,
                   enable_asserts=False, num_devices=cfg.ncores)

    # ---- I/O
    x0T = nc.dram_tensor("x0T", [fin1, S], f32, kind="ExternalInput")
    x0s = nc.dram_tensor("x0s", [S, fin1], f32, kind="ExternalInput")
    gsrc = nc.dram_tensor("gsrc", [128, T], i32, kind="ExternalInput")
    dstrel = nc.dram_tensor("dstrel", [128, T], bf16, kind="ExternalInput")
    wext_d, bias_d = [], []
    for li, (fi, fo, h) in enumerate(cfg.layers):
        wext_d.append(nc.dram_tensor(f"Wext{li + 1}", [fi, _xcols(fo, h)], f32,
                                     kind="ExternalInput"))
        bias_d.append(nc.dram_tensor(f"bias{li + 1}", [1, fo], f32,
                                     kind="ExternalInput"))
    final_out = nc.dram_tensor("final_out", [S, 64], f32, kind="ExternalOutput")
    if dump:
        dmp = {
            "d_iota": nc.dram_tensor("d_iota", [128, 128], f32, kind="ExternalOutput"),
            "d_xsl": nc.dram_tensor("d_xsl", [S, _xcols(cfg.layers[0][1], cfg.layers[0][2])], f32, kind="ExternalOutput"),
            "d_xfull": nc.dram_tensor("d_xfull", [NP, _xcols(cfg.layers[0][1], cfg.layers[0][2])], f32, kind="ExternalOutput"),
            "d_gbuf": nc.dram_tensor("d_gbuf", [128, cfg.mcap * _xcols(cfg.layers[0][1], cfg.layers[0][2])], f32, kind="ExternalOutput"),
            "d_sel": nc.dram_tensor("d_sel", [128, 128], f32, kind="ExternalOutput"),
            "d_selT": nc.dram_tensor("d_selT", [128, 128], f32, kind="ExternalOutput"),
            "d_psb": nc.dram_tensor("d_psb", [128, cfg.mcap * cfg.layers[0][2]], f32, kind="ExternalOutput"),
            "d_ops": nc.dram_tensor("d_ops", [128, cfg.layers[0][1] + cfg.layers[0][2]], f32, kind="ExternalOutput"),
            "d_adst": nc.dram_tensor("d_adst", [128, cfg.layers[0][2]], f32, kind="ExternalOutput"),
            "d_adeps": nc.dram_tensor("d_adeps", [128, cfg.mcap * cfg.layers[0][2]], f32, kind="ExternalOutput"),
        }
    alpha_out = [nc.dram_tensor(f"alpha{li + 1}", [128, T * h], f32,
                                kind="ExternalOutput")
                 for li, (_, _, h) in enumerate(cfg.layers)]

    rg = [list(range(cfg.ncores))]

    with tile.TileContext(nc) as tc:
        with tc.tile_pool(name="const", bufs=1) as constp, \
             tc.tile_pool(name="resident", bufs=1) as resp, \
             tc.tile_pool(name="dram", bufs=1, space="DRAM") as dramp:

            # ---- constants
            identity = constp.tile([128, 128], bf16, name="identity")
            make_identity(nc, identity[:])
            identity_f = constp.tile([128, 128], f32, name="identity_f")
            make_identity(nc, identity_f[:])
            iota_i32 = constp.tile([128, 128], i32, name="iota_i32")
            nc.gpsimd.iota(iota_i32[:], pattern=[[1, 128]], base=0,
                           channel_multiplier=0)
            iota_row = constp.tile([128, 128], bf16, name="iota_row")
            nc.vector.tensor_copy(iota_row[:], iota_i32[:])
            if dump:
                nc.sync.dma_start(dmp["d_iota"][:, :], iota_row[:])

            # ---- resident edge metadata
            gsrc_sb = resp.tile([128, T], i32, name="gsrc_sb")
            nc.sync.dma_start(gsrc_sb[:], gsrc[:, :])
            dstrel_sb = resp.tile([128, T], bf16, name="dstrel_sb")
            nc.sync.dma_start(dstrel_sb[:], dstrel[:, :])

            # ---- big DRAM scratch
            xsl_d = dramp.tile([S * XM], bf16, name="xsl_d")
            xfull_ds = [dramp.tile([NP * _xcols(fo_, h_)], bf16,
                                   name=f"xfull_d{li_}", addr_space="Shared")
                        for li_, (_, fo_, h_) in enumerate(cfg.layers)]
            xT_d = dramp.tile([128, S], f32, name="xT_d")      # relu(x) transposed
            out_d = [dramp.tile([S, fo], f32, name=f"out_d{li}")
                     for li, (_, fo, _) in enumerate(cfg.layers)]

            for li, (fi, fo, h) in enumerate(cfg.layers):
                xc = _xcols(fo, h)
                glast = li == len(cfg.layers) - 1
                xsl_v = xsl_d[:].flatten()[:S * xc].rearrange("(n c) -> n c", c=xc)
                xfull_flat = xfull_ds[li][:].flatten()
                xfull_v = xfull_flat[:NP * xc].rearrange("(n c) -> n c", c=xc)

                # ---- bias broadcast tile
                with tc.tile_pool(name=f"bias{li}", bufs=1) as biasp:
                    bias_bc = biasp.tile([128, fo], f32, name=f"bias_bc{li}")
                    nc.sync.dma_start(bias_bc[:1, :], bias_d[li][:, :])
                    nc.gpsimd.partition_broadcast(bias_bc[:], bias_bc[:1, :])

                    # ================= node phase =================
                    with tc.tile_pool(name="np_w", bufs=1) as wp, \
                         tc.tile_pool(name="np_lhs", bufs=3) as lhsp, \
                         tc.tile_pool(name="np_ps", bufs=2, space="PSUM") as npp, \
                         tc.tile_pool(name="np_st", bufs=3) as stp:
                        wt = wp.tile([fi, xc], f32, name=f"wt{li}")
                        nc.sync.dma_start(wt[:], wext_d[li][:, :])
                        adst_res = resp.tile([128, B * h], bf16,
                                             name=f"adst_res{li}")
                        for b in range(B):
                            lhs = lhsp.tile([fi, 128], f32, tag="lhs")
                            if li == 0:
                                nc.sync.dma_start(
                                    lhs[:], x0T[:, b * 128:(b + 1) * 128])
                            else:
                                nc.sync.dma_start(
                                    lhs[:], xT_d[:, b * 128:(b + 1) * 128])
                            ps = npp.tile([128, xc], f32, tag="nps")
                            nc.tensor.matmul(ps[:], lhsT=lhs[:], rhs=wt[:],
                                             start=True, stop=True)
                            st = stp.tile([128, xc], bf16, tag="nst")
                            nc.scalar.copy(st[:], ps[:])
                            nc.vector.tensor_copy(
                                adst_res[:, b * h:(b + 1) * h],
                                ps[:, fo + h:fo + 2 * h])
                            nc.sync.dma_start(
                                xsl_v[b * 128:(b + 1) * 128, :], st[:])

                    # ================= allgather =================
                    if dump and li == 0:
                        nc.sync.dma_start(dmp["d_xsl"][:, :], xsl_v[:, :])
                    nc.gpsimd.collective_compute(
                        "AllGather", OP.bypass, replica_groups=rg,
                        ins=[xsl_d[:].flatten()[:S * xc]],
                        outs=[xfull_flat[:NP * xc]])

                    if dump and li == 0:
                        nc.sync.dma_start(dmp["d_xfull"][:, :], xfull_v[:, :])
                    # ================= edge phase =================
                    with tc.tile_pool(name="eg_g", bufs=4) as gp, \
                         tc.tile_pool(name="eg_sel", bufs=2) as selp, \
                         tc.tile_pool(name="eg_selT", bufs=2) as selTp, \
                         tc.tile_pool(name="eg_trps", bufs=2, space="PSUM") as trp, \
                         tc.tile_pool(name="eg_adps", bufs=2, space="PSUM") as adpsp, \
                         tc.tile_pool(name="eg_alps", bufs=2, space="PSUM") as alpsp, \
                         tc.tile_pool(name="eg_ops", bufs=2, space="PSUM") as opsp, \
                         tc.tile_pool(name="eg_sm", bufs=3) as smp, \
                         tc.tile_pool(name="eg_rhs", bufs=2) as rhsp, \
                         tc.tile_pool(name="eg_ep", bufs=2) as epp:
                        for b in range(B):
                            gb = gp.tile([128, M * xc], bf16, tag="gbuf")
                            for t in range(M):
                                g = b * M + t
                                nc.gpsimd.indirect_dma_start(
                                    out=gb[:, t * xc:(t + 1) * xc],
                                    out_offset=None,
                                    in_=xfull_v,
                                    in_offset=bass.IndirectOffsetOnAxis(
                                        ap=gsrc_sb[:, g:g + 1], axis=0))
                            gb3 = gb[:].rearrange("p (t c) -> p t c", c=xc)
                            if dump and li == 0 and b == 0:
                                nc.sync.dma_start(dmp["d_gbuf"][:, :], gb[:])
                                nc.sync.dma_start(dmp["d_adst"][:, :],
                                                  adst_res[:, :h])

                            adstblk = adst_res[:, b * h:(b + 1) * h]
                            sel = selp.tile([128, M * 128], bf16, tag="sel")
                            selT = selTp.tile([128, M * 128], bf16, tag="selT")
                            ade_ps = adpsp.tile([128, M * h], f32, tag="adps")
                            for t in range(M):
                                g = b * M + t
                                sl = sel[:, t * 128:(t + 1) * 128]
                                nc.vector.tensor_tensor(
                                    out=sl, in0=iota_row[:],
                                    in1=dstrel_sb[:, g:g + 1].to_broadcast(
                                        [128, 128]),
                                    op=OP.is_equal)
                                tp = trp.tile([128, 128], bf16, tag="trps")
                                nc.tensor.transpose(tp[:], sl, identity[:])
                                sT = selT[:, t * 128:(t + 1) * 128]
                                nc.scalar.copy(sT, tp[:])
                                nc.tensor.matmul(
                                    ade_ps[:, t * h:(t + 1) * h],
                                    lhsT=sT, rhs=adstblk,
                                    start=True, stop=True)
                                if dump and li == 0 and b == 0 and t == 0:
                                    nc.sync.dma_start(dmp["d_sel"][:, :], sl)
                                    nc.sync.dma_start(dmp["d_selT"][:, :], sT)

                            # batched per-block softmax numerators
                            e_sb = smp.tile([128, M * h], f32, tag="esb")
                            nc.vector.tensor_tensor(
                                out=e_sb[:].rearrange("p (t h) -> p t h", h=h),
                                in0=ade_ps[:].rearrange("p (t h) -> p t h", h=h),
                                in1=gb3[:, :, fo:fo + h],
                                op=OP.add)
                            lr_sb = smp.tile([128, M * h], f32, tag="lrsb")
                            nc.vector.scalar_tensor_tensor(
                                out=lr_sb[:], in0=e_sb[:], scalar=NEG_SLOPE,
                                in1=e_sb[:], op0=OP.mult, op1=OP.max)
                            p_sb = smp.tile([128, M * h], f32, tag="psb")
                            nc.scalar.activation(p_sb[:], lr_sb[:], AT.Exp)
                            p3 = p_sb[:].rearrange("p (t h) -> p t h", h=h)
                            if dump and li == 0 and b == 0:
                                nc.sync.dma_start(dmp["d_psb"][:, :], p_sb[:])
                                nc.sync.dma_start(dmp["d_adeps"][:, :], e_sb[:])

                            # rhs = [p * xw | p]
                            rhs = rhsp.tile([128, M * (fo + h)], bf16, tag="rhs")
                            rhs3 = rhs[:].rearrange("p (t c) -> p t c", c=fo + h)
                            c = fo // h
                            for hh in range(h):
                                nc.vector.tensor_tensor(
                                    out=rhs3[:, :, hh * c:(hh + 1) * c],
                                    in0=gb3[:, :, hh * c:(hh + 1) * c],
                                    in1=p3[:, :, hh:hh + 1].to_broadcast(
                                        [128, M, c]),
                                    op=OP.mult)
                            nc.vector.tensor_copy(rhs3[:, :, fo:fo + h], p3)

                            # main accumulation
                            ops = opsp.tile([128, fo + h], f32, tag="ops")
                            for t in range(M):
                                nc.tensor.matmul(
                                    ops[:],
                                    lhsT=sel[:, t * 128:(t + 1) * 128],
                                    rhs=rhs3[:, t, :],
                                    start=(t == 0), stop=(t == M - 1))

                            # epilogue: normalize, bias, store
                            den = epp.tile([128, h], f32, tag="den")
                            nc.vector.tensor_scalar(den[:], ops[:, fo:fo + h],
                                                    1e-16, None, op0=OP.add)
                            rden = epp.tile([128, h], f32, tag="rden")
                            nc.vector.reciprocal(rden[:], den[:])
                            rdenb = epp.tile([128, h], bf16, tag="rdenb")
                            nc.vector.tensor_copy(rdenb[:], rden[:])
                            xb = epp.tile([128, fo], f32, tag="xb")
                            nc.vector.tensor_tensor(
                                out=xb[:].rearrange("p (h c) -> p h c", h=h),
                                in0=ops[:, :fo].rearrange("p (h c) -> p h c", h=h),
                                in1=rden[:].unsqueeze(2).to_broadcast([128, h, c]),
                                op=OP.mult)
                            xbb = epp.tile([128, fo], f32, tag="xbb")
                            nc.vector.tensor_tensor(out=xbb[:], in0=xb[:],
                                                    in1=bias_bc[:], op=OP.add)
                            if dump and li == 0 and b == 0:
                                nc.sync.dma_start(
                                    dmp["d_ops"][:, :fo], xbb[:])
                            nc.sync.dma_start(
                                out_d[li][b * 128:(b + 1) * 128, :], xbb[:])
                            if not glast:
                                xr = epp.tile([128, fo], f32, tag="xr")
                                nc.scalar.activation(xr[:], xbb[:], AT.Relu)
                                xrt = trp.tile([128, 128], f32, tag="trps")
                                nc.tensor.transpose(xrt[:, :fo], xr[:],
                                                    identity_f[:])
                                xrs = epp.tile([128, 128], f32, tag="xrs")
                                nc.vector.tensor_copy(xrs[:fo, :], xrt[:fo, :])
                                nc.sync.dma_start(
                                    xT_d[:fo, b * 128:(b + 1) * 128],
                                    xrs[:fo, :])

                            # pass B: alpha = p * (selT @ rden)
                            al_ps = alpsp.tile([128, M * h], f32, tag="alps")
                            for t in range(M):
                                nc.tensor.matmul(
                                    al_ps[:, t * h:(t + 1) * h],
                                    lhsT=selT[:, t * 128:(t + 1) * 128],
                                    rhs=rdenb[:], start=True, stop=True)
                            al_sb = smp.tile([128, M * h], f32, tag="alsb")
                            nc.vector.tensor_tensor(out=al_sb[:], in0=al_ps[:],
                                                    in1=p_sb[:], op=OP.mult)
                            nc.sync.dma_start(
                                alpha_out[li][:, b * M * h:(b + 1) * M * h],
                                al_sb[:])

                            # final combine fused into last layer's epilogue
                            if glast:
                                f0 = epp.tile([128, 64], f32, tag="f0")
                                o1 = epp.tile([128, 128], f32, tag="o1")
                                o2 = epp.tile([128, 128], f32, tag="o2")
                                nc.sync.dma_start(
                                    f0[:], x0s[b * 128:(b + 1) * 128, :])
                                nc.sync.dma_start(
                                    o1[:], out_d[0][b * 128:(b + 1) * 128, :])
                                nc.sync.dma_start(
                                    o2[:], out_d[1][b * 128:(b + 1) * 128, :])
                                t1 = epp.tile([128, 64], f32, tag="t1")
                                nc.vector.tensor_tensor(
                                    out=t1[:], in0=o1[:, :64], in1=o1[:, 64:],
                                    op=OP.add)
                                t2 = epp.tile([128, 64], f32, tag="t2")
                                nc.vector.tensor_tensor(
                                    out=t2[:], in0=o2[:, :64], in1=o2[:, 64:],
                                    op=OP.add)
                                s1 = epp.tile([128, 64], f32, tag="s1")
                                nc.vector.scalar_tensor_tensor(
                                    out=s1[:], in0=t1[:], scalar=0.5,
                                    in1=f0[:], op0=OP.mult, op1=OP.add)
                                s2 = epp.tile([128, 64], f32, tag="s2")
                                nc.vector.scalar_tensor_tensor(
                                    out=s2[:], in0=t2[:], scalar=0.5,
                                    in1=s1[:], op0=OP.mult, op1=OP.add)
                                s3 = epp.tile([128, 64], f32, tag="s3")
                                nc.vector.tensor_tensor(
                                    out=s3[:], in0=s2[:], in1=xbb[:],
                                    op=OP.add)
                                fin = epp.tile([128, 64], f32, tag="fin")
                                nc.vector.tensor_scalar(
                                    fin[:], s3[:], 0.25, None, op0=OP.mult)
                                nc.sync.dma_start(
                                    final_out[b * 128:(b + 1) * 128, :],
                                    fin[:])

    nc.compile()
    return nc


# ---------------------------------------------------------------- entry

_CACHED = {}


def _run_on_hw(in_maps):
    sys.path.insert(0, "/opt/trn_rl_repo")
    from concourse import bass_utils
    if "nc" not in _CACHED:
        _CACHED["nc"] = build_nc()
    nc = _CACHED["nc"]
    kw = {}
    if os.environ.get("GAT_TRACE"):
        sys.path.insert(0, "/root/problem")
        import axon_prof
        axon_prof.install()
        kw = dict(trace=True, tmpdir=os.environ.get("GAT_TRACE"),
                  trace_cores=[int(os.environ.get("GAT_TRACE_CORE", "0"))])
    res = bass_utils.run_bass_kernel_spmd(
        nc, in_maps, core_ids=list(range(NCORES)), **kw)
    if res.exec_time_ns is not None:
        print(f"HW exec time: {res.exec_time_ns} ns")
    return res.results


def kernel(**inputs):
    inputs = {k: np.asarray(v) for k, v in inputs.items()}
    in_maps, origs = host_prep_all(inputs)
    results = _run_on_hw(in_maps)
    return postprocess(results, origs)
